# revision 1
# baseline (speedup 1.0000x reference)
"""Cross-attention (Bahdanau-style) scores kernel for 8 Trainium2 NeuronCores.

Reference computation (per batch b, source position s):
    energy[b,s,:] = tanh(Wh @ h[b] + We @ eo[s,b] + bias)
    scores[b,s]   = v . energy[b,s,:]
    out[b,:]      = softmax(scores[b,:])   over s

Sharding: data-parallel over batch (64 batches -> 8 per core). Weights are
replicated. No collectives needed (softmax is per-batch, fully local).

Per-core pipeline (S=4096, Bc=8, E2=512, D=256):
  - DMA natural tiles eo[s0:s0+512, bb, :] as [p=128, st=4, e=512] (1 MiB/DMA)
  - PE transposes [s128, e128] -> PSUM [e128, s128] (e onto partitions)
  - ACT/DVE copy PSUM->SBUF building eoT chunks [e128, s512]
  - PE matmul (float32r): eprojT[k128, s512] += WeT[e,k].T @ eoT[e,s]
  - ACT: energy = tanh(eprojT + baseT[k]) fused (per-partition bias, PSUM in)
  - PE dot: scores[1, s512] += v[k].T @ energy[k, s]
  - batched softmax over all 8 batches at the end ([8, 4096] tiles)
"""

import numpy as np
import ml_dtypes

import concourse.bass as bass
import concourse.bacc as bacc
import concourse.tile as tile
from concourse import mybir
from concourse.bass_utils import run_bass_kernel_spmd

dt = mybir.dt

S = 4096          # src_len
B = 64            # global batch
E2 = 512          # 2*enc_hid
D = 256           # dec_hid
NCORES = 8
BC = B // NCORES  # batches per core = 8
P = 128
SG = 512          # s-group size
NG = S // SG      # 8 s-groups
NST = SG // P     # 4 s-subtiles per group
NEC = E2 // P     # 4 e-chunks
NKC = D // P      # 2 k-chunks

F32 = dt.float32
F32R = dt.float32r
BF16 = dt.bfloat16

# f32r: 1 cycle/row matmul (vs 4 for plain f32) when out free dim >= 256.
PROJ_DT = F32R
TRANS_DT = F32R   # transpose: f32r 1.5 cyc/row vs f32 2.0


def _r(ap, d):
    """bitcast an AP's dtype (same element size)."""
    return ap.bitcast(d) if d is not None else ap


def build_program():
    nc = bacc.Bacc(None, target_bir_lowering=False, debug=False, num_devices=8)

    # eoT[bb, c, p, s] = eo[s, bb, c*128+p]  (host pre-transposed, bf16)
    eoT_d = nc.declare_dram_parameter("eoT", [BC, NEC, P, S], BF16, isOutput=False)
    # WeT_r[p, ec, k] = We.T[ec*128+p, k] ; We = W[:, D:]
    weT_d = nc.declare_dram_parameter("weT", [P, NEC, D], BF16, isOutput=False)
    # WhT_r[p, dc, kc, j] = W[kc*128+j, dc*128+p]  (Wh part, pre-chunked)
    whT_d = nc.declare_dram_parameter("whT", [P, NKC, NKC, P], F32, isOutput=False)
    # hT[p, dc, bb] = h[bb, dc*128+p]
    hT_d = nc.declare_dram_parameter("hT", [P, NKC, BC], F32, isOutput=False)
    # bT[p, kc] = bias[kc*128+p]
    bT_d = nc.declare_dram_parameter("bT", [P, NKC], F32, isOutput=False)
    # vT[p, kc] = v[kc*128+p]
    # vm[p, kc, bb, m] = v[kc*128+p] if m == bb else 0  (dot -> partition bb)
    vm_d = nc.declare_dram_parameter("vm", [P, NKC, BC, BC], BF16, isOutput=False)
    out_d = nc.declare_dram_parameter("out", [BC, S], F32, isOutput=True)

    with tile.TileContext(nc) as tc:
        with tc.tile_pool(name="consts", bufs=1) as consts:
            weT = consts.tile([P, NEC, D], BF16)
            nc.sync.dma_start(out=weT, in_=weT_d[:])
            vm = consts.tile([P, NKC, BC, BC], BF16)
            nc.sync.dma_start(out=vm, in_=vm_d[:])
            bT = consts.tile([P, NKC], F32)
            nc.sync.dma_start(out=bT, in_=bT_d[:])
            whT = consts.tile([P, NKC, NKC, P], F32)
            nc.sync.dma_start(out=whT, in_=whT_d[:])
            hT = consts.tile([P, NKC, BC], F32)
            nc.sync.dma_start(out=hT, in_=hT_d[:])

            baseT = consts.tile([P, NKC, BC], F32)   # [k128, kc, bb]
            esums = consts.tile([BC, NG], F32)       # per-group exp sums
            out_sb = consts.tile([BC, S], F32)

            # --- init: baseT[k, bb] = sum_d Wh[k, d] h[bb, d] + bias[k] ---
            with tc.tile_pool(name="initps", bufs=1, space="PSUM") as initps:
                ps_base = initps.tile([P, NKC, BC], F32)
                for kc in range(NKC):
                    for dc in range(NKC):
                        nc.tensor.matmul(
                            ps_base[:, kc, :],
                            whT[:, dc, kc, :],
                            hT[:, dc, :],
                            start=(dc == 0),
                            stop=(dc == NKC - 1),
                        )
                for kc in range(NKC):
                    nc.vector.tensor_scalar_add(
                        baseT[:, kc, :], ps_base[:, kc, :], bT[:, kc : kc + 1]
                    )

            with (
                tc.tile_pool(name="eot", bufs=8) as eot_pool,
                tc.tile_pool(name="en", bufs=6) as en_pool,
                tc.tile_pool(name="pep", bufs=6, space="PSUM") as pep_pool,
                tc.tile_pool(name="psc", bufs=2, space="PSUM") as psc_pool,
            ):
                for g in range(NG):
                    s0 = g * SG
                    ps_sc = psc_pool.tile([BC, SG], F32, tag="psc")
                    for bb in range(BC):
                        # ---- load pre-transposed tile [p, c, s] (512 KiB) ----
                        eoT_t = eot_pool.tile([P, NEC, SG], BF16, tag="eot")
                        nc.sync.dma_start(
                            out=eoT_t,
                            in_=eoT_d[bb, :, :, s0 : s0 + SG].rearrange(
                                "c p j -> p c j"
                            ),
                        )

                        # ---- projection + tanh(+bias) + dot ----
                        for kc in range(NKC):
                            ps_ep = pep_pool.tile([P, SG], F32, tag="pep")
                            for c in range(NEC):
                                nc.tensor.matmul(
                                    ps_ep,
                                    weT[:, c, kc * P : (kc + 1) * P],
                                    eoT_t[:, c, :],
                                    start=(c == 0),
                                    stop=(c == NEC - 1),
                                )
                            en = en_pool.tile([P, SG], BF16, tag="en")
                            nc.scalar.activation(
                                out=en, in_=ps_ep,
                                func=mybir.ActivationFunctionType.Tanh,
                                bias=baseT[:, kc, bb : bb + 1],
                            )
                            nc.tensor.matmul(
                                ps_sc,
                                vm[:, kc, bb, :],
                                en,
                                start=(bb == 0 and kc == 0),
                                stop=(bb == BC - 1 and kc == NKC - 1),
                            )
                    nc.scalar.activation(
                        out=out_sb[:, s0 : s0 + SG], in_=ps_sc,
                        func=mybir.ActivationFunctionType.Exp,
                        accum_out=esums[:, g : g + 1],
                    )

                # ---- softmax tail: combine per-group sums, scale ----
                with tc.tile_pool(name="sm", bufs=1) as sm:
                    esum = sm.tile([BC, 1], F32)
                    nc.vector.tensor_reduce(
                        out=esum, in_=esums, axis=mybir.AxisListType.X,
                        op=mybir.AluOpType.add,
                    )
                    rsum = sm.tile([BC, 1], F32)
                    nc.vector.reciprocal(rsum, esum)
                    nc.scalar.activation(
                        out=out_sb, in_=out_sb,
                        func=mybir.ActivationFunctionType.Copy,
                        scale=rsum,
                    )
                    nc.sync.dma_start(out=out_d[:], in_=out_sb)

    return nc


_nc = None


def _get_nc():
    global _nc
    if _nc is None:
        _nc = build_program()
        _nc.compile()
    return _nc


def kernel(hidden, encoder_outputs, W, b, v):
    hidden = np.asarray(hidden, dtype=np.float32)
    encoder_outputs = np.ascontiguousarray(encoder_outputs, dtype=np.float32)
    W = np.asarray(W, dtype=np.float32)
    b = np.asarray(b, dtype=np.float32)
    v = np.asarray(v, dtype=np.float32)

    # host-side prep of the small replicated weights
    We = W[:, D:]                                     # [256, 512]
    weT = np.ascontiguousarray(
        We.T.reshape(NEC, P, D).transpose(1, 0, 2)    # [p, ec, k]
    ).astype(ml_dtypes.bfloat16)
    # whT[p, dc, kc, j] = W[kc*128+j, dc*128+p]
    Wh = W[:, :D]                                     # [k, d]
    whT = np.ascontiguousarray(
        Wh.reshape(NKC, P, NKC, P).transpose(3, 2, 0, 1)  # [p(d), dc, kc, j(k)]
    )
    bT = np.ascontiguousarray(b.reshape(NKC, P).T)    # [p, kc]
    vT = np.ascontiguousarray(v.reshape(NKC, P).T)
    vm = np.zeros((P, NKC, BC, BC), dtype=np.float32)
    for bb in range(BC):
        vm[:, :, bb, bb] = vT
    vm = vm.astype(ml_dtypes.bfloat16)
    h = hidden[0]                                     # [64, 256]

    nc = _get_nc()
    eo_bf16 = encoder_outputs.astype(ml_dtypes.bfloat16)
    # [S, B, E2] -> [B, E2, S], then per-core slice reshapes to [BC, NEC, P, S]
    eoT_full = np.ascontiguousarray(eo_bf16.transpose(1, 2, 0))
    in_maps = []
    for i in range(NCORES):
        bsl = slice(i * BC, (i + 1) * BC)
        hT_i = np.ascontiguousarray(h[bsl].T.reshape(NKC, P, BC).transpose(1, 0, 2))
        eoT_i = np.ascontiguousarray(eoT_full[bsl]).reshape(BC, NEC, P, S)
        in_maps.append(
            {"eoT": eoT_i, "weT": weT, "whT": whT, "hT": hT_i, "bT": bT,
             "vm": vm}
        )

    try:
        res = run_bass_kernel_spmd(nc, in_maps, list(range(NCORES)))
    except Exception:
        # transient NRT/device hiccups happen; one retry
        res = run_bass_kernel_spmd(nc, in_maps, list(range(NCORES)))
    global _last_results
    _last_results = res
    out = np.concatenate([res.results[i]["out"] for i in range(NCORES)], axis=0)
    return out


_last_results = None


if __name__ == "__main__":
    rng = np.random.default_rng(0)
    inputs = {
        "hidden": rng.standard_normal((1, B, D), dtype=np.float32),
        "encoder_outputs": rng.standard_normal((S, B, E2), dtype=np.float32),
        "W": (rng.standard_normal((D, E2 + D)) * 0.02).astype(np.float32),
        "b": (rng.standard_normal((D,)) * 0.02).astype(np.float32),
        "v": rng.random((D,), dtype=np.float32),
    }
    out = kernel(**inputs)
    print("out", out.shape, out.dtype, out.sum())



# revision 4
# speedup vs baseline: 1.2615x; 1.2615x over previous
"""Cross-attention (Bahdanau-style) scores kernel for 8 Trainium2 NeuronCores.

Reference computation (per batch b, source position s):
    energy[b,s,:] = tanh(Wh @ h[b] + We @ eo[s,b] + bias)
    scores[b,s]   = v . energy[b,s,:]
    out[b,:]      = softmax(scores[b,:])   over s

Sharding: data-parallel over batch (64 batches -> 8 per core), weights
replicated, no collectives.

Per-core pipeline (S=4096, Bc=8, E2=512, D=256), v2:
  - host pre-transposes eo to [NG, 4, P(e), 2(bb), NEC, SG] bf16 so every
    DMA is 1 MiB with 8 KiB contiguous per partition (128 descriptors).
  - PE warm-up: dummy matmuls during the DMA preamble keep HAM at 8/8.
  - projection: weT chunk stationary, eoT moving (N=512), psum ring.
  - ACT: energy = tanh(proj + baseT[k]) fused per-partition bias, -> bf16.
  - dot: v . energy via 4-wide col-tiled matmuls (tile_position), all 16
    (bb,kc) dots of a group accumulate into one PSUM bank at partition
    32*(bb%4) + bb//4.  Packs are emitted one (h,kc) stage late so the PE
    queue never stalls waiting on ACT.
  - exp (accum_out per-group sums) -> bf16, per-group DMA out.
  - softmax normalization happens on host (divide by gathered sums).
"""

import numpy as np
import ml_dtypes

import concourse.bass as bass
import concourse.bacc as bacc
import concourse.tile as tile
from concourse import mybir
from concourse.bass_utils import run_bass_kernel_spmd

dt = mybir.dt

S = 4096          # src_len
B = 64            # global batch
E2 = 512          # 2*enc_hid
D = 256           # dec_hid
NCORES = 8
BC = B // NCORES  # batches per core = 8
P = 128
SG = 512          # s-group size
NG = S // SG      # 8 s-groups
NEC = E2 // P     # 4 e-chunks
NKC = D // P      # 2 k-chunks
NQ = 4            # 1 MiB dma chunks per s-group (2 batches each)

F32 = dt.float32
BF16 = dt.bfloat16

N_WARMUP_MM = 18  # dummy matmuls bridging the DMA preamble (HAM warm-up)


def build_program():
    nc = bacc.Bacc(None, target_bir_lowering=False, debug=False, num_devices=8)

    # eoT[g, q, p, r, c, j] = eo[g*SG+j, 2q+r (local), c*128+p]  (bf16)
    eoT_d = nc.declare_dram_parameter("eoT", [NG, NQ, P, 2, NEC, SG], BF16,
                                      isOutput=False)
    # weT[p, ec, k] = We.T[ec*128+p, k] ; We = W[:, D:]
    weT_d = nc.declare_dram_parameter("weT", [P, NEC, D], BF16, isOutput=False)
    # whT[p, dc, kc, j] = W[kc*128+j, dc*128+p]  (Wh part, pre-chunked)
    whT_d = nc.declare_dram_parameter("whT", [P, NKC, NKC, P], F32, isOutput=False)
    # hT[p, dc, bb] = h[bb, dc*128+p]
    hT_d = nc.declare_dram_parameter("hT", [P, NKC, BC], F32, isOutput=False)
    # bT[p, kc] = bias[kc*128+p]
    bT_d = nc.declare_dram_parameter("bT", [P, NKC], F32, isOutput=False)
    # vm[p, kc, h, m] = v[kc*128+p] if m == h else 0   (col-tiled dot weights)
    vm_d = nc.declare_dram_parameter("vm", [P, NKC, 2, 32], BF16, isOutput=False)
    # outputs: raw exp(scores) rows live at partition 32*(bb%4) + bb//4
    out_d = nc.declare_dram_parameter("out", [P, S], BF16, isOutput=True)
    esums_d = nc.declare_dram_parameter("esums", [P, NG], F32, isOutput=True)

    with tile.TileContext(nc) as tc:
        # ---- PE warm-up: keep HAM busy while DMAs stream in ----
        with (
            tc.tile_pool(name="warm", bufs=1) as wp,
            tc.tile_pool(name="warmps", bufs=1, space="PSUM") as wpp,
        ):
            dmy = wp.tile([P, SG], BF16)
            nc.vector.memset(dmy, 0)
            wps = wpp.tile([P, SG], F32)
            for _ in range(N_WARMUP_MM):
                nc.tensor.matmul(wps, dmy[:, :P], dmy, start=True, stop=True)

        with tc.tile_pool(name="consts", bufs=1) as consts:
            weT = consts.tile([P, NEC, D], BF16)
            nc.sync.dma_start(out=weT, in_=weT_d[:])
            whT = consts.tile([P, NKC, NKC, P], F32)
            nc.sync.dma_start(out=whT, in_=whT_d[:])
            hT = consts.tile([P, NKC, BC], F32)
            nc.sync.dma_start(out=hT, in_=hT_d[:])
            bT = consts.tile([P, NKC], F32)
            nc.sync.dma_start(out=bT, in_=bT_d[:])
            vm = consts.tile([P, NKC, 2, 32], BF16)
            nc.sync.dma_start(out=vm, in_=vm_d[:])

            baseT = consts.tile([P, NKC, BC], F32)   # [k128, kc, bb]
            esums = consts.tile([P, NG], F32)        # per-group exp sums
            out_sb = consts.tile([P, S], BF16)       # exp(scores), scattered rows

            # --- init: baseT[k, bb] = sum_d Wh[k, d] h[bb, d] + bias[k] ---
            with tc.tile_pool(name="initps", bufs=1, space="PSUM") as initps:
                ps_base = initps.tile([P, NKC, BC], F32)
                for kc in range(NKC):
                    for dc in range(NKC):
                        nc.tensor.matmul(
                            ps_base[:, kc, :],
                            whT[:, dc, kc, :],
                            hT[:, dc, :],
                            start=(dc == 0),
                            stop=(dc == NKC - 1),
                        )
                for kc in range(NKC):
                    nc.vector.tensor_scalar_add(
                        baseT[:, kc, :], ps_base[:, kc, :], bT[:, kc : kc + 1]
                    )

            with (
                tc.tile_pool(name="eot", bufs=8) as eot_pool,
                tc.tile_pool(name="en", bufs=9) as en_pool,
                tc.tile_pool(name="pep", bufs=6, space="PSUM") as pep_pool,
                tc.tile_pool(name="psc", bufs=2, space="PSUM") as psc_pool,
            ):
                pending_pack = []   # deferred dot-MM emission (one stage late)
                pending_exp = []    # deferred exp emission for finished group

                def flush_pack():
                    while pending_pack:
                        pending_pack.pop(0)()
                    while pending_exp:
                        pending_exp.pop(0)()

                for g in range(NG):
                    s0 = g * SG
                    ps_sc = psc_pool.tile([P, SG], F32, tag="psc")
                    # 1 MiB chunk DMAs for this group
                    eo_t = []
                    for q in range(NQ):
                        t = eot_pool.tile([P, 2, NEC, SG], BF16, tag="eot")
                        nc.sync.dma_start(out=t, in_=eoT_d[g, q])
                        eo_t.append(t)

                    for h in range(2):
                        for kc in range(NKC):
                            # ---- projection: 16 MMs, weT chunk stationary ----
                            pss = []
                            for _pi in range(4):
                                ps_ep = pep_pool.tile([P, SG], F32, tag="pep")
                                pss.append(ps_ep)
                            for b2 in range(4):
                                bb = 4 * h + b2
                                for c in range(NEC):
                                    nc.tensor.matmul(
                                        pss[b2],
                                        weT[:, c, kc * P : (kc + 1) * P],
                                        eo_t[bb // 2][:, bb % 2, c, :],
                                        start=(c == 0),
                                        stop=(c == NEC - 1),
                                    )
                            # emit the previous stage's dot pack now: its ACT
                            # deps are long done, so the PE queue never stalls
                            flush_pack()
                            # ---- tanh(+bias) -> bf16 energy ----
                            ens = []
                            for b2 in range(4):
                                bb = 4 * h + b2
                                en = en_pool.tile([P, SG], BF16, tag="en")
                                nc.scalar.activation(
                                    out=en, in_=pss[b2],
                                    func=mybir.ActivationFunctionType.Tanh,
                                    bias=baseT[:, kc, bb : bb + 1],
                                )
                                ens.append(en)

                            def make_pack(ps_sc=ps_sc, ens=ens, h=h, kc=kc):
                                def emit():
                                    for b2 in range(4):
                                        nc.tensor.matmul(
                                            ps_sc[32 * b2 : 32 * b2 + 32, :],
                                            vm[:, kc, h, :],
                                            ens[b2],
                                            start=(h == 0 and kc == 0),
                                            stop=(h == 1 and kc == NKC - 1),
                                            tile_position=(0, 32 * b2),
                                            skip_group_check=True,
                                        )
                                return emit
                            pending_pack.append(make_pack())

                    def make_exp(ps_sc=ps_sc, g=g, s0=s0):
                        def emit():
                            nc.scalar.activation(
                                out=out_sb[:, s0 : s0 + SG], in_=ps_sc,
                                func=mybir.ActivationFunctionType.Exp,
                                accum_out=esums[:, g : g + 1],
                            )
                            nc.sync.dma_start(
                                out=out_d[:, s0 : s0 + SG],
                                in_=out_sb[:, s0 : s0 + SG],
                            )
                        return emit
                    pending_exp.append(make_exp())

                flush_pack()
                nc.sync.dma_start(out=esums_d[:], in_=esums)

    return nc


_nc = None


def _get_nc():
    global _nc
    if _nc is None:
        _nc = build_program()
        _nc.compile()
    return _nc


def kernel(hidden, encoder_outputs, W, b, v):
    hidden = np.asarray(hidden, dtype=np.float32)
    encoder_outputs = np.ascontiguousarray(encoder_outputs, dtype=np.float32)
    W = np.asarray(W, dtype=np.float32)
    b = np.asarray(b, dtype=np.float32)
    v = np.asarray(v, dtype=np.float32)

    # host-side prep of the small replicated weights
    We = W[:, D:]                                     # [256, 512]
    weT = np.ascontiguousarray(
        We.T.reshape(NEC, P, D).transpose(1, 0, 2)    # [p, ec, k]
    ).astype(ml_dtypes.bfloat16)
    Wh = W[:, :D]                                     # [k, d]
    whT = np.ascontiguousarray(
        Wh.reshape(NKC, P, NKC, P).transpose(3, 2, 0, 1)  # [p(d), dc, kc, j(k)]
    )
    bT = np.ascontiguousarray(b.reshape(NKC, P).T)    # [p, kc]
    vT = np.ascontiguousarray(v.reshape(NKC, P).T)    # [p, kc]
    vm = np.zeros((P, NKC, 2, 32), dtype=np.float32)
    for h in range(2):
        vm[:, :, h, h] = vT
    vm = vm.astype(ml_dtypes.bfloat16)
    h_ = hidden[0]                                    # [64, 256]

    nc = _get_nc()
    eo_bf16 = encoder_outputs.astype(ml_dtypes.bfloat16)
    # [S, B, E2] -> [NG, SG, NC, NQ, 2, NEC, P]
    eo_r = eo_bf16.reshape(NG, SG, NCORES, NQ, 2, NEC, P)
    in_maps = []
    for i in range(NCORES):
        bsl = slice(i * BC, (i + 1) * BC)
        hT_i = np.ascontiguousarray(
            h_[bsl].T.reshape(NKC, P, BC).transpose(1, 0, 2))
        # per-core: [NG, SG, NQ, 2, NEC, P] -> [NG, NQ, P, 2, NEC, SG]
        eoT_i = np.ascontiguousarray(eo_r[:, :, i].transpose(0, 2, 5, 3, 4, 1))
        in_maps.append(
            {"eoT": eoT_i, "weT": weT, "whT": whT, "hT": hT_i, "bT": bT,
             "vm": vm}
        )

    try:
        res = run_bass_kernel_spmd(nc, in_maps, list(range(NCORES)))
    except Exception:
        # transient NRT/device hiccups happen; one retry
        res = run_bass_kernel_spmd(nc, in_maps, list(range(NCORES)))
    global _last_results
    _last_results = res

    out = np.empty((B, S), dtype=np.float32)
    for i in range(NCORES):
        exps = np.asarray(res.results[i]["out"]).astype(np.float32)   # [128, S]
        sums = np.asarray(res.results[i]["esums"]).astype(np.float64)  # [128, NG]
        for bb in range(BC):
            row = 32 * (bb % 4) + bb // 4
            denom = np.float32(sums[row].sum())
            out[i * BC + bb] = exps[row] / denom
    return out


_last_results = None


if __name__ == "__main__":
    rng = np.random.default_rng(0)
    inputs = {
        "hidden": rng.standard_normal((1, B, D), dtype=np.float32),
        "encoder_outputs": rng.standard_normal((S, B, E2), dtype=np.float32),
        "W": (rng.standard_normal((D, E2 + D)) * 0.02).astype(np.float32),
        "b": (rng.standard_normal((D,)) * 0.02).astype(np.float32),
        "v": rng.random((D,), dtype=np.float32),
    }
    out = kernel(**inputs)
    print("out", out.shape, out.dtype, out.sum())


# revision 8
# speedup vs baseline: 1.2643x; 1.0022x over previous
"""Cross-attention (Bahdanau-style) scores kernel for 8 Trainium2 NeuronCores.

Reference computation (per batch b, source position s):
    energy[b,s,:] = tanh(Wh @ h[b] + We @ eo[s,b] + bias)
    scores[b,s]   = v . energy[b,s,:]
    out[b,:]      = softmax(scores[b,:])   over s

Sharding: data-parallel over batch (64 batches -> 8 per core), weights
replicated, no collectives.

Per-core pipeline (S=4096, Bc=8, E2=512, D=256), v2:
  - host pre-transposes eo to [NG, 4, P(e), 2(bb), NEC, SG] bf16 so every
    DMA is 1 MiB with 8 KiB contiguous per partition (128 descriptors).
  - PE warm-up: dummy matmuls during the DMA preamble keep HAM at 8/8.
  - projection: weT chunk stationary, eoT moving (N=512), psum ring.
  - ACT: energy = tanh(proj + baseT[k]) fused per-partition bias, -> bf16.
  - dot: v . energy via 4-wide col-tiled matmuls (tile_position), all 16
    (bb,kc) dots of a group accumulate into one PSUM bank at partition
    32*(bb%4) + bb//4.  Packs are emitted one (h,kc) stage late so the PE
    queue never stalls waiting on ACT.
  - exp (accum_out per-group sums) -> bf16, per-group DMA out.
  - softmax normalization happens on host (divide by gathered sums).
"""

import numpy as np
import ml_dtypes

import concourse.bass as bass
import concourse.bacc as bacc
import concourse.tile as tile
from concourse import mybir
from concourse.bass_utils import run_bass_kernel_spmd

dt = mybir.dt

S = 4096          # src_len
B = 64            # global batch
E2 = 512          # 2*enc_hid
D = 256           # dec_hid
NCORES = 8
BC = B // NCORES  # batches per core = 8
P = 128
SG = 512          # s-group size
NG = S // SG      # 8 s-groups
NEC = E2 // P     # 4 e-chunks
NKC = D // P      # 2 k-chunks
NQ = 4            # 1 MiB dma chunks per s-group (2 batches each)

F32 = dt.float32
BF16 = dt.bfloat16

N_WARMUP_MM = 9   # dummy matmuls bridging the DMA preamble (HAM warm-up)


def build_program():
    nc = bacc.Bacc(None, target_bir_lowering=False, debug=False, num_devices=8)

    # eoT[g, q, p, r, c, j] = eo[g*SG+j, 2q+r (local), c*128+p]  (bf16)
    eoT_d = nc.declare_dram_parameter("eoT", [NG, NQ, P, 2, NEC, SG], BF16,
                                      isOutput=False)
    # weT[p, ec, k] = We.T[ec*128+p, k] ; We = W[:, D:]
    weT_d = nc.declare_dram_parameter("weT", [P, NEC, D], BF16, isOutput=False)
    # whT[p, dc, kc, j] = W[kc*128+j, dc*128+p]  (Wh part, pre-chunked)
    whT_d = nc.declare_dram_parameter("whT", [P, NKC, NKC, P], F32, isOutput=False)
    # hT[p, dc, bb] = h[bb, dc*128+p]
    hT_d = nc.declare_dram_parameter("hT", [P, NKC, BC], F32, isOutput=False)
    # bT[p, kc] = bias[kc*128+p]
    bT_d = nc.declare_dram_parameter("bT", [P, NKC], F32, isOutput=False)
    # vm[p, kc, h, m] = v[kc*128+p] if m == h else 0   (col-tiled dot weights)
    vm_d = nc.declare_dram_parameter("vm", [P, NKC, 2, 32], BF16, isOutput=False)
    # outputs: raw exp(scores) rows live at partition 32*(bb%4) + bb//4
    out_d = nc.declare_dram_parameter("out", [P, S], BF16, isOutput=True)
    esums_d = nc.declare_dram_parameter("esums", [P, NG], F32, isOutput=True)

    with tile.TileContext(nc) as tc:
        # warm pools stay OPEN for the whole program: closing them would let
        # later pools reuse their SBUF/PSUM and create false deps that block
        # the chunk DMAs behind the warm-up matmuls (cost 10us in v2).
        with (
            tc.tile_pool(name="warm", bufs=1) as wp,
            tc.tile_pool(name="warmps", bufs=1, space="PSUM") as wpp,
            tc.tile_pool(name="consts", bufs=1) as consts,
        ):
            # ---- PE warm-up: keep HAM busy while DMAs stream in ----
            dmy = wp.tile([P, SG], BF16)
            nc.vector.memset(dmy, 0)
            wps = wpp.tile([P, SG], F32)
            for _ in range(N_WARMUP_MM):
                nc.tensor.matmul(wps, dmy[:, :P], dmy, start=True, stop=True)

            # consts ride the second HWDGE ring (ACT engine) so they stream
            # concurrently with the first eo chunks on the sync ring.
            weT = consts.tile([P, NEC, D], BF16)
            nc.scalar.dma_start(out=weT, in_=weT_d[:])
            whT = consts.tile([P, NKC, NKC, P], F32)
            nc.scalar.dma_start(out=whT, in_=whT_d[:])
            hT = consts.tile([P, NKC, BC], F32)
            nc.scalar.dma_start(out=hT, in_=hT_d[:])
            # dummy activation: forces the ACT table set (tanh+exp) to load
            # during the preamble instead of before the first real tanh.
            dmy_act = wp.tile([P, SG], BF16)
            nc.scalar.activation(
                out=dmy_act, in_=dmy,
                func=mybir.ActivationFunctionType.Tanh,
            )
            bT = consts.tile([P, NKC], F32)
            nc.scalar.dma_start(out=bT, in_=bT_d[:])
            vm = consts.tile([P, NKC, 2, 32], BF16)
            nc.scalar.dma_start(out=vm, in_=vm_d[:])

            baseT = consts.tile([P, NKC, BC], F32)   # [k128, kc, bb]
            esums = consts.tile([P, NG], F32)        # per-group exp sums
            out_sb = consts.tile([P, S], BF16)       # exp(scores), scattered rows

            # --- init: baseT[k, bb] = sum_d Wh[k, d] h[bb, d] + bias[k] ---
            with tc.tile_pool(name="initps", bufs=1, space="PSUM") as initps:
                ps_base = initps.tile([P, NKC, BC], F32)
                for kc in range(NKC):
                    for dc in range(NKC):
                        nc.tensor.matmul(
                            ps_base[:, kc, :],
                            whT[:, dc, kc, :],
                            hT[:, dc, :],
                            start=(dc == 0),
                            stop=(dc == NKC - 1),
                        )
                for kc in range(NKC):
                    nc.vector.tensor_scalar_add(
                        baseT[:, kc, :], ps_base[:, kc, :], bT[:, kc : kc + 1]
                    )

            with (
                tc.tile_pool(name="eot", bufs=8) as eot_pool,
                tc.tile_pool(name="en", bufs=9) as en_pool,
                tc.tile_pool(name="pep", bufs=5, space="PSUM") as pep_pool,
                tc.tile_pool(name="psc", bufs=2, space="PSUM") as psc_pool,
            ):
                pending_pack = []   # deferred dot-MM emission (one stage late)
                pending_exp = []    # deferred exp emission for finished group

                def flush_pack():
                    while pending_pack:
                        pending_pack.pop(0)()
                    while pending_exp:
                        pending_exp.pop(0)()

                for g in range(NG):
                    s0 = g * SG
                    ps_sc = psc_pool.tile([P, SG], F32, tag="psc")
                    # 1 MiB chunk DMAs for this group
                    eo_t = []
                    for q in range(NQ):
                        t = eot_pool.tile([P, 2, NEC, SG], BF16, tag="eot")
                        nc.sync.dma_start(out=t, in_=eoT_d[g, q])
                        eo_t.append(t)

                    for h in range(2):
                        for kc in range(NKC):
                            # ---- projection: 16 MMs, weT chunk stationary ----
                            pss = []
                            for _pi in range(4):
                                ps_ep = pep_pool.tile([P, SG], F32, tag="pep")
                                pss.append(ps_ep)
                            for b2 in range(4):
                                bb = 4 * h + b2
                                for c in range(NEC):
                                    nc.tensor.matmul(
                                        pss[b2],
                                        weT[:, c, kc * P : (kc + 1) * P],
                                        eo_t[bb // 2][:, bb % 2, c, :],
                                        start=(c == 0),
                                        stop=(c == NEC - 1),
                                    )
                            # emit the previous stage's dot pack now: its ACT
                            # deps are long done, so the PE queue never stalls
                            flush_pack()
                            # ---- tanh(+bias) -> bf16 energy ----
                            ens = []
                            for b2 in range(4):
                                bb = 4 * h + b2
                                en = en_pool.tile([P, SG], BF16, tag="en")
                                nc.scalar.activation(
                                    out=en, in_=pss[b2],
                                    func=mybir.ActivationFunctionType.Tanh,
                                    bias=baseT[:, kc, bb : bb + 1],
                                )
                                ens.append(en)

                            def make_pack(ps_sc=ps_sc, ens=ens, h=h, kc=kc):
                                def emit():
                                    for b2 in range(4):
                                        nc.tensor.matmul(
                                            ps_sc[32 * b2 : 32 * b2 + 32, :],
                                            vm[:, kc, h, :],
                                            ens[b2],
                                            start=(h == 0 and kc == 0),
                                            stop=(h == 1 and kc == NKC - 1),
                                            tile_position=(0, 32 * b2),
                                            skip_group_check=True,
                                        )
                                return emit
                            pending_pack.append(make_pack())

                    def make_exp(ps_sc=ps_sc, g=g, s0=s0):
                        def emit():
                            nc.scalar.activation(
                                out=out_sb[:, s0 : s0 + SG], in_=ps_sc,
                                func=mybir.ActivationFunctionType.Exp,
                                accum_out=esums[:, g : g + 1],
                            )
                            nc.sync.dma_start(
                                out=out_d[:, s0 : s0 + SG],
                                in_=out_sb[:, s0 : s0 + SG],
                            )
                        return emit
                    pending_exp.append(make_exp())

                flush_pack()
                nc.scalar.dma_start(out=esums_d[:], in_=esums)

    return nc


_nc = None


def _get_nc():
    global _nc
    if _nc is None:
        _nc = build_program()
        _nc.compile()
    return _nc


def kernel(hidden, encoder_outputs, W, b, v):
    hidden = np.asarray(hidden, dtype=np.float32)
    encoder_outputs = np.ascontiguousarray(encoder_outputs, dtype=np.float32)
    W = np.asarray(W, dtype=np.float32)
    b = np.asarray(b, dtype=np.float32)
    v = np.asarray(v, dtype=np.float32)

    # host-side prep of the small replicated weights
    We = W[:, D:]                                     # [256, 512]
    weT = np.ascontiguousarray(
        We.T.reshape(NEC, P, D).transpose(1, 0, 2)    # [p, ec, k]
    ).astype(ml_dtypes.bfloat16)
    Wh = W[:, :D]                                     # [k, d]
    whT = np.ascontiguousarray(
        Wh.reshape(NKC, P, NKC, P).transpose(3, 2, 0, 1)  # [p(d), dc, kc, j(k)]
    )
    bT = np.ascontiguousarray(b.reshape(NKC, P).T)    # [p, kc]
    vT = np.ascontiguousarray(v.reshape(NKC, P).T)    # [p, kc]
    vm = np.zeros((P, NKC, 2, 32), dtype=np.float32)
    for h in range(2):
        vm[:, :, h, h] = vT
    vm = vm.astype(ml_dtypes.bfloat16)
    h_ = hidden[0]                                    # [64, 256]

    nc = _get_nc()
    eo_bf16 = encoder_outputs.astype(ml_dtypes.bfloat16)
    # [S, B, E2] -> [NG, SG, NC, NQ, 2, NEC, P]
    eo_r = eo_bf16.reshape(NG, SG, NCORES, NQ, 2, NEC, P)
    in_maps = []
    for i in range(NCORES):
        bsl = slice(i * BC, (i + 1) * BC)
        hT_i = np.ascontiguousarray(
            h_[bsl].T.reshape(NKC, P, BC).transpose(1, 0, 2))
        # per-core: [NG, SG, NQ, 2, NEC, P] -> [NG, NQ, P, 2, NEC, SG]
        eoT_i = np.ascontiguousarray(eo_r[:, :, i].transpose(0, 2, 5, 3, 4, 1))
        in_maps.append(
            {"eoT": eoT_i, "weT": weT, "whT": whT, "hT": hT_i, "bT": bT,
             "vm": vm}
        )

    try:
        res = run_bass_kernel_spmd(nc, in_maps, list(range(NCORES)))
    except Exception:
        # transient NRT/device hiccups happen; one retry
        res = run_bass_kernel_spmd(nc, in_maps, list(range(NCORES)))
    global _last_results
    _last_results = res

    out = np.empty((B, S), dtype=np.float32)
    for i in range(NCORES):
        exps = np.asarray(res.results[i]["out"]).astype(np.float32)   # [128, S]
        sums = np.asarray(res.results[i]["esums"]).astype(np.float64)  # [128, NG]
        for bb in range(BC):
            row = 32 * (bb % 4) + bb // 4
            denom = np.float32(sums[row].sum())
            out[i * BC + bb] = exps[row] / denom
    return out


_last_results = None


if __name__ == "__main__":
    rng = np.random.default_rng(0)
    inputs = {
        "hidden": rng.standard_normal((1, B, D), dtype=np.float32),
        "encoder_outputs": rng.standard_normal((S, B, E2), dtype=np.float32),
        "W": (rng.standard_normal((D, E2 + D)) * 0.02).astype(np.float32),
        "b": (rng.standard_normal((D,)) * 0.02).astype(np.float32),
        "v": rng.random((D,), dtype=np.float32),
    }
    out = kernel(**inputs)
    print("out", out.shape, out.dtype, out.sum())


# revision 13
# speedup vs baseline: 1.2936x; 1.0232x over previous
"""Cross-attention (Bahdanau-style) scores kernel for 8 Trainium2 NeuronCores.

Reference computation (per batch b, source position s):
    energy[b,s,:] = tanh(Wh @ h[b] + We @ eo[s,b] + bias)
    scores[b,s]   = v . energy[b,s,:]
    out[b,:]      = softmax(scores[b,:])   over s

Sharding: data-parallel over batch (64 batches -> 8 per core), weights
replicated, no collectives.

Per-core pipeline (S=4096, Bc=8, E2=512, D=256), v2:
  - host pre-transposes eo to [NG, 4, P(e), 2(bb), NEC, SG] bf16 so every
    DMA is 1 MiB with 8 KiB contiguous per partition (128 descriptors).
  - PE warm-up: dummy matmuls during the DMA preamble keep HAM at 8/8.
  - projection: weT chunk stationary, eoT moving (N=512), psum ring.
  - ACT: energy = tanh(proj + baseT[k]) fused per-partition bias, -> bf16.
  - dot: v . energy via 4-wide col-tiled matmuls (tile_position), all 16
    (bb,kc) dots of a group accumulate into one PSUM bank at partition
    32*(bb%4) + bb//4.  Packs are emitted one (h,kc) stage late so the PE
    queue never stalls waiting on ACT.
  - exp (accum_out per-group sums) -> bf16, per-group DMA out.
  - softmax normalization happens on host (divide by gathered sums).
"""

import numpy as np
import ml_dtypes

import concourse.bass as bass
import concourse.bacc as bacc
import concourse.tile as tile
from concourse import mybir
from concourse.bass_utils import run_bass_kernel_spmd

dt = mybir.dt

S = 4096          # src_len
B = 64            # global batch
E2 = 512          # 2*enc_hid
D = 256           # dec_hid
NCORES = 8
BC = B // NCORES  # batches per core = 8
P = 128
SG = 512          # s-group size
NG = S // SG      # 8 s-groups
NEC = E2 // P     # 4 e-chunks
NKC = D // P      # 2 k-chunks
NQ = 4            # 1 MiB dma chunks per s-group (2 batches each)

F32 = dt.float32
BF16 = dt.bfloat16

N_WARMUP_MM = 10  # dummy matmuls bridging the DMA preamble (HAM warm-up)


def build_program():
    nc = bacc.Bacc(None, target_bir_lowering=False, debug=False, num_devices=8)

    # eoT[g, q, p, r, c, j] = eo[g*SG+j, 2q+r (local), c*128+p]  (bf16)
    eoT_d = nc.declare_dram_parameter("eoT", [NG, NQ, P, 2, NEC, SG], BF16,
                                      isOutput=False)
    # weT[p, ec, k] = We.T[ec*128+p, k] ; We = W[:, D:]
    weT_d = nc.declare_dram_parameter("weT", [P, NEC, D], BF16, isOutput=False)
    # baseT[p, kc, bb] = (Wh @ h[bb] + b)[kc*128+p]  (host-precomputed)
    baseT_d = nc.declare_dram_parameter("baseT", [P, NKC, BC], F32, isOutput=False)
    # vm[p, kc, h, m] = v[kc*128+p] if m == h else 0   (col-tiled dot weights)
    vm_d = nc.declare_dram_parameter("vm", [P, NKC, 2, 32], BF16, isOutput=False)
    # outputs: raw exp(scores) rows live at partition 32*(bb%4) + bb//4
    out_d = nc.declare_dram_parameter("out", [P, S], BF16, isOutput=True)
    esums_d = nc.declare_dram_parameter("esums", [P, NG], F32, isOutput=True)

    with tile.TileContext(nc) as tc:
        # warm pools stay OPEN for the whole program: closing them would let
        # later pools reuse their SBUF/PSUM and create false deps that block
        # the chunk DMAs behind the warm-up matmuls (cost 10us in v2).
        with (
            tc.tile_pool(name="warm", bufs=1) as wp,
            tc.tile_pool(name="warmps", bufs=1, space="PSUM") as wpp,
            tc.tile_pool(name="consts", bufs=1) as consts,
        ):
            # ---- PE warm-up: keep HAM busy while DMAs stream in ----
            dmy = wp.tile([P, SG], BF16)
            nc.vector.memset(dmy, 0)
            wps = wpp.tile([P, SG], F32)
            for _ in range(N_WARMUP_MM):
                nc.tensor.matmul(wps, dmy[:, :P], dmy, start=True, stop=True)

            # consts ride the second HWDGE ring (ACT engine) so they stream
            # concurrently with the first eo chunks on the sync ring.
            weT = consts.tile([P, NEC, D], BF16)
            nc.scalar.dma_start(out=weT, in_=weT_d[:])
            baseT = consts.tile([P, NKC, BC], F32)   # [k128, kc, bb]
            nc.scalar.dma_start(out=baseT, in_=baseT_d[:])
            # dummy activation: forces the ACT table set (tanh+exp) to load
            # during the preamble instead of before the first real tanh.
            dmy_act = wp.tile([P, SG], BF16)
            nc.scalar.activation(
                out=dmy_act, in_=dmy,
                func=mybir.ActivationFunctionType.Tanh,
            )
            vm = consts.tile([P, NKC, 2, 32], BF16)
            nc.scalar.dma_start(out=vm, in_=vm_d[:])

            esums = consts.tile([P, NG], F32)        # per-group exp sums
            out_sb = consts.tile([P, S], BF16)       # exp(scores), scattered rows

            with (
                tc.tile_pool(name="eot", bufs=8) as eot_pool,
                tc.tile_pool(name="en", bufs=9) as en_pool,
                tc.tile_pool(name="pep", bufs=5, space="PSUM") as pep_pool,
                tc.tile_pool(name="psc", bufs=2, space="PSUM") as psc_pool,
            ):
                pending_pack = []   # deferred dot-MM emission (one stage late)
                pending_exp = []    # deferred exp emission for finished group

                def flush_pack():
                    while pending_pack:
                        pending_pack.pop(0)()
                    while pending_exp:
                        pending_exp.pop(0)()

                for g in range(NG):
                    s0 = g * SG
                    ps_sc = psc_pool.tile([P, SG], F32, tag="psc")
                    # 1 MiB chunk DMAs for this group
                    eo_t = []
                    for q in range(NQ):
                        t = eot_pool.tile([P, 2, NEC, SG], BF16, tag="eot")
                        nc.sync.dma_start(out=t, in_=eoT_d[g, q])
                        eo_t.append(t)

                    for h in range(2):
                        for kc in range(NKC):
                            # ---- projection: 16 MMs, weT chunk stationary ----
                            pss = []
                            for _pi in range(4):
                                ps_ep = pep_pool.tile([P, SG], F32, tag="pep")
                                pss.append(ps_ep)
                            for b2 in range(4):
                                bb = 4 * h + b2
                                for c in range(NEC):
                                    nc.tensor.matmul(
                                        pss[b2],
                                        weT[:, c, kc * P : (kc + 1) * P],
                                        eo_t[bb // 2][:, bb % 2, c, :],
                                        start=(c == 0),
                                        stop=(c == NEC - 1),
                                    )
                            # emit the previous stage's dot pack now: its ACT
                            # deps are long done, so the PE queue never stalls
                            flush_pack()
                            # ---- tanh(+bias) -> bf16 energy ----
                            ens = []
                            for b2 in range(4):
                                bb = 4 * h + b2
                                en = en_pool.tile([P, SG], BF16, tag="en")
                                nc.scalar.activation(
                                    out=en, in_=pss[b2],
                                    func=mybir.ActivationFunctionType.Tanh,
                                    bias=baseT[:, kc, bb : bb + 1],
                                )
                                ens.append(en)

                            def make_pack(ps_sc=ps_sc, ens=ens, h=h, kc=kc):
                                def emit():
                                    for b2 in range(4):
                                        nc.tensor.matmul(
                                            ps_sc[32 * b2 : 32 * b2 + 32, :],
                                            vm[:, kc, h, :],
                                            ens[b2],
                                            start=(h == 0 and kc == 0),
                                            stop=(h == 1 and kc == NKC - 1),
                                            tile_position=(0, 32 * b2),
                                            skip_group_check=True,
                                        )
                                return emit
                            pending_pack.append(make_pack())

                    def make_exp(ps_sc=ps_sc, g=g, s0=s0):
                        def emit():
                            nc.scalar.activation(
                                out=out_sb[:, s0 : s0 + SG], in_=ps_sc,
                                func=mybir.ActivationFunctionType.Exp,
                                accum_out=esums[:, g : g + 1],
                            )
                            nc.sync.dma_start(
                                out=out_d[:, s0 : s0 + SG],
                                in_=out_sb[:, s0 : s0 + SG],
                            )
                        return emit
                    pending_exp.append(make_exp())

                flush_pack()
                nc.scalar.dma_start(out=esums_d[:], in_=esums)

    return nc


_nc = None


def _get_nc():
    global _nc
    if _nc is None:
        _nc = build_program()
        _nc.compile()
    return _nc


def kernel(hidden, encoder_outputs, W, b, v):
    hidden = np.asarray(hidden, dtype=np.float32)
    encoder_outputs = np.ascontiguousarray(encoder_outputs, dtype=np.float32)
    W = np.asarray(W, dtype=np.float32)
    b = np.asarray(b, dtype=np.float32)
    v = np.asarray(v, dtype=np.float32)

    # host-side prep of the small replicated weights
    We = W[:, D:]                                     # [256, 512]
    weT = np.ascontiguousarray(
        We.T.reshape(NEC, P, D).transpose(1, 0, 2)    # [p, ec, k]
    ).astype(ml_dtypes.bfloat16)
    Wh = W[:, :D]                                     # [k, d]
    vT = np.ascontiguousarray(v.reshape(NKC, P).T)    # [p, kc]
    vm = np.zeros((P, NKC, 2, 32), dtype=np.float32)
    for h in range(2):
        vm[:, :, h, h] = vT
    vm = vm.astype(ml_dtypes.bfloat16)
    h_ = hidden[0]                                    # [64, 256]

    nc = _get_nc()
    eo_bf16 = encoder_outputs.astype(ml_dtypes.bfloat16)
    # [S, B, E2] -> [NG, SG, NC, NQ, 2, NEC, P]
    eo_r = eo_bf16.reshape(NG, SG, NCORES, NQ, 2, NEC, P)
    in_maps = []
    for i in range(NCORES):
        bsl = slice(i * BC, (i + 1) * BC)
        # baseT[p, kc, bb] = (h @ Wh.T + b)[bb, kc*128+p]
        base_i = (h_[bsl] @ Wh.T + b).astype(np.float32)      # [BC, D]
        baseT_i = np.ascontiguousarray(
            base_i.T.reshape(NKC, P, BC).transpose(1, 0, 2))
        # per-core: [NG, SG, NQ, 2, NEC, P] -> [NG, NQ, P, 2, NEC, SG]
        eoT_i = np.ascontiguousarray(eo_r[:, :, i].transpose(0, 2, 5, 3, 4, 1))
        in_maps.append(
            {"eoT": eoT_i, "weT": weT, "baseT": baseT_i, "vm": vm}
        )

    try:
        res = run_bass_kernel_spmd(nc, in_maps, list(range(NCORES)))
    except Exception:
        # transient NRT/device hiccups happen; one retry
        res = run_bass_kernel_spmd(nc, in_maps, list(range(NCORES)))
    global _last_results
    _last_results = res

    out = np.empty((B, S), dtype=np.float32)
    for i in range(NCORES):
        exps = np.asarray(res.results[i]["out"]).astype(np.float32)   # [128, S]
        sums = np.asarray(res.results[i]["esums"]).astype(np.float64)  # [128, NG]
        for bb in range(BC):
            row = 32 * (bb % 4) + bb // 4
            denom = np.float32(sums[row].sum())
            out[i * BC + bb] = exps[row] / denom
    return out


_last_results = None


if __name__ == "__main__":
    rng = np.random.default_rng(0)
    inputs = {
        "hidden": rng.standard_normal((1, B, D), dtype=np.float32),
        "encoder_outputs": rng.standard_normal((S, B, E2), dtype=np.float32),
        "W": (rng.standard_normal((D, E2 + D)) * 0.02).astype(np.float32),
        "b": (rng.standard_normal((D,)) * 0.02).astype(np.float32),
        "v": rng.random((D,), dtype=np.float32),
    }
    out = kernel(**inputs)
    print("out", out.shape, out.dtype, out.sum())


# revision 14
# speedup vs baseline: 1.3126x; 1.0147x over previous
"""Cross-attention (Bahdanau-style) scores kernel for 8 Trainium2 NeuronCores.

Reference computation (per batch b, source position s):
    energy[b,s,:] = tanh(Wh @ h[b] + We @ eo[s,b] + bias)
    scores[b,s]   = v . energy[b,s,:]
    out[b,:]      = softmax(scores[b,:])   over s

Sharding: data-parallel over batch (64 batches -> 8 per core), weights
replicated, no collectives.

Per-core pipeline (S=4096, Bc=8, E2=512, D=256), v2:
  - host pre-transposes eo to [NG, 4, P(e), 2(bb), NEC, SG] bf16 so every
    DMA is 1 MiB with 8 KiB contiguous per partition (128 descriptors).
  - PE warm-up: dummy matmuls during the DMA preamble keep HAM at 8/8.
  - projection: weT chunk stationary, eoT moving (N=512), psum ring.
  - ACT: energy = tanh(proj + baseT[k]) fused per-partition bias, -> bf16.
  - dot: v . energy via 4-wide col-tiled matmuls (tile_position), all 16
    (bb,kc) dots of a group accumulate into one PSUM bank at partition
    32*(bb%4) + bb//4.  Packs are emitted one (h,kc) stage late so the PE
    queue never stalls waiting on ACT.
  - exp (accum_out per-group sums) -> bf16, per-group DMA out.
  - softmax normalization happens on host (divide by gathered sums).
"""

import numpy as np
import ml_dtypes

import concourse.bass as bass
import concourse.bacc as bacc
import concourse.tile as tile
from concourse import mybir
from concourse.bass_utils import run_bass_kernel_spmd

dt = mybir.dt

S = 4096          # src_len
B = 64            # global batch
E2 = 512          # 2*enc_hid
D = 256           # dec_hid
NCORES = 8
BC = B // NCORES  # batches per core = 8
P = 128
SG = 512          # s-group size
NG = S // SG      # 8 s-groups
NEC = E2 // P     # 4 e-chunks
NKC = D // P      # 2 k-chunks
NQ = 4            # 1 MiB dma chunks per s-group (2 batches each)

F32 = dt.float32
BF16 = dt.bfloat16

N_WARMUP_MM = 16  # dummy matmuls bridging the DMA preamble (HAM warm-up)


def build_program():
    nc = bacc.Bacc(None, target_bir_lowering=False, debug=False, num_devices=8)

    # eoT[g, q, p, r, c, j] = eo[g*SG+j, 2q+r (local), c*128+p]  (bf16)
    eoT_d = nc.declare_dram_parameter("eoT", [NG, NQ, P, 2, NEC, SG], BF16,
                                      isOutput=False)
    # weT[p, ec, k] = We.T[ec*128+p, k] ; We = W[:, D:]
    weT_d = nc.declare_dram_parameter("weT", [P, NEC, D], BF16, isOutput=False)
    # baseT[p, kc, bb] = (Wh @ h[bb] + b)[kc*128+p]  (host-precomputed)
    baseT_d = nc.declare_dram_parameter("baseT", [P, NKC, BC], F32, isOutput=False)
    # vm[p, kc, h, m] = v[kc*128+p] if m == h else 0   (col-tiled dot weights)
    vm_d = nc.declare_dram_parameter("vm", [P, NKC, 2, 32], BF16, isOutput=False)
    # outputs: raw exp(scores) rows live at partition 32*(bb%4) + bb//4
    out_d = nc.declare_dram_parameter("out", [P, S], BF16, isOutput=True)
    esums_d = nc.declare_dram_parameter("esums", [P, NG], F32, isOutput=True)

    with tile.TileContext(nc) as tc:
        # warm pools stay OPEN for the whole program: closing them would let
        # later pools reuse their SBUF/PSUM and create false deps that block
        # the chunk DMAs behind the warm-up matmuls (cost 10us in v2).
        with (
            tc.tile_pool(name="warm", bufs=1) as wp,
            tc.tile_pool(name="warmps", bufs=1, space="PSUM") as wpp,
            tc.tile_pool(name="consts", bufs=1) as consts,
        ):
            # ---- PE warm-up: keep HAM busy while DMAs stream in ----
            dmy = wp.tile([P, SG], BF16)
            nc.vector.memset(dmy, 0)
            wps = wpp.tile([P, SG], F32)
            for _ in range(N_WARMUP_MM):
                nc.tensor.matmul(wps, dmy[:, :P], dmy, start=True, stop=True)

            # consts ride the second HWDGE ring (ACT engine) so they stream
            # concurrently with the first eo chunks on the sync ring.
            weT = consts.tile([P, NEC, D], BF16)
            nc.scalar.dma_start(out=weT, in_=weT_d[:])
            baseT = consts.tile([P, NKC, BC], F32)   # [k128, kc, bb]
            nc.scalar.dma_start(out=baseT, in_=baseT_d[:])
            # dummy activation: forces the ACT table set (tanh+exp) to load
            # during the preamble instead of before the first real tanh.
            dmy_act = wp.tile([P, SG], BF16)
            nc.scalar.activation(
                out=dmy_act, in_=dmy,
                func=mybir.ActivationFunctionType.Tanh,
            )
            vm = consts.tile([P, NKC, 2, 32], BF16)
            nc.scalar.dma_start(out=vm, in_=vm_d[:])

            esums = consts.tile([P, NG], F32)        # per-group exp sums
            out_sb = consts.tile([P, S], BF16)       # exp(scores), scattered rows

            with (
                tc.tile_pool(name="eot", bufs=8) as eot_pool,
                tc.tile_pool(name="en", bufs=9) as en_pool,
                tc.tile_pool(name="pep", bufs=5, space="PSUM") as pep_pool,
                tc.tile_pool(name="psc", bufs=2, space="PSUM") as psc_pool,
            ):
                pending_pack = []   # deferred dot-MM emission (one stage late)
                pending_exp = []    # deferred exp emission for finished group

                def flush_pack():
                    while pending_pack:
                        pending_pack.pop(0)()
                    while pending_exp:
                        pending_exp.pop(0)()

                for g in range(NG):
                    s0 = g * SG
                    ps_sc = psc_pool.tile([P, SG], F32, tag="psc")
                    # 1 MiB chunk DMAs for this group
                    eo_t = []
                    for q in range(NQ):
                        t = eot_pool.tile([P, 2, NEC, SG], BF16, tag="eot")
                        nc.sync.dma_start(out=t, in_=eoT_d[g, q])
                        eo_t.append(t)

                    for h in range(2):
                        for kc in range(NKC):
                            # ---- projection: 16 MMs, weT chunk stationary ----
                            pss = []
                            for _pi in range(4):
                                ps_ep = pep_pool.tile([P, SG], F32, tag="pep")
                                pss.append(ps_ep)
                            for b2 in range(4):
                                bb = 4 * h + b2
                                for c in range(NEC):
                                    nc.tensor.matmul(
                                        pss[b2],
                                        weT[:, c, kc * P : (kc + 1) * P],
                                        eo_t[bb // 2][:, bb % 2, c, :],
                                        start=(c == 0),
                                        stop=(c == NEC - 1),
                                    )
                            # emit the previous stage's dot pack now: its ACT
                            # deps are long done, so the PE queue never stalls
                            flush_pack()
                            # ---- tanh(+bias) -> bf16 energy ----
                            ens = []
                            for b2 in range(4):
                                bb = 4 * h + b2
                                en = en_pool.tile([P, SG], BF16, tag="en")
                                nc.scalar.activation(
                                    out=en, in_=pss[b2],
                                    func=mybir.ActivationFunctionType.Tanh,
                                    bias=baseT[:, kc, bb : bb + 1],
                                )
                                ens.append(en)

                            def make_pack(ps_sc=ps_sc, ens=ens, h=h, kc=kc):
                                def emit():
                                    for b2 in range(4):
                                        nc.tensor.matmul(
                                            ps_sc[32 * b2 : 32 * b2 + 32, :],
                                            vm[:, kc, h, :],
                                            ens[b2],
                                            start=(h == 0 and kc == 0),
                                            stop=(h == 1 and kc == NKC - 1),
                                            tile_position=(0, 32 * b2),
                                            skip_group_check=True,
                                        )
                                return emit
                            pending_pack.append(make_pack())

                    def make_exp(ps_sc=ps_sc, g=g, s0=s0):
                        def emit():
                            nc.scalar.activation(
                                out=out_sb[:, s0 : s0 + SG], in_=ps_sc,
                                func=mybir.ActivationFunctionType.Exp,
                                accum_out=esums[:, g : g + 1],
                            )
                            nc.sync.dma_start(
                                out=out_d[:, s0 : s0 + SG],
                                in_=out_sb[:, s0 : s0 + SG],
                            )
                        return emit
                    pending_exp.append(make_exp())

                flush_pack()
                nc.scalar.dma_start(out=esums_d[:], in_=esums)

    return nc


_nc = None


def _get_nc():
    global _nc
    if _nc is None:
        _nc = build_program()
        _nc.compile()
    return _nc


def kernel(hidden, encoder_outputs, W, b, v):
    hidden = np.asarray(hidden, dtype=np.float32)
    encoder_outputs = np.ascontiguousarray(encoder_outputs, dtype=np.float32)
    W = np.asarray(W, dtype=np.float32)
    b = np.asarray(b, dtype=np.float32)
    v = np.asarray(v, dtype=np.float32)

    # host-side prep of the small replicated weights
    We = W[:, D:]                                     # [256, 512]
    weT = np.ascontiguousarray(
        We.T.reshape(NEC, P, D).transpose(1, 0, 2)    # [p, ec, k]
    ).astype(ml_dtypes.bfloat16)
    Wh = W[:, :D]                                     # [k, d]
    vT = np.ascontiguousarray(v.reshape(NKC, P).T)    # [p, kc]
    vm = np.zeros((P, NKC, 2, 32), dtype=np.float32)
    for h in range(2):
        vm[:, :, h, h] = vT
    vm = vm.astype(ml_dtypes.bfloat16)
    h_ = hidden[0]                                    # [64, 256]

    nc = _get_nc()
    eo_bf16 = encoder_outputs.astype(ml_dtypes.bfloat16)
    # [S, B, E2] -> [NG, SG, NC, NQ, 2, NEC, P]
    eo_r = eo_bf16.reshape(NG, SG, NCORES, NQ, 2, NEC, P)
    in_maps = []
    for i in range(NCORES):
        bsl = slice(i * BC, (i + 1) * BC)
        # baseT[p, kc, bb] = (h @ Wh.T + b)[bb, kc*128+p]
        base_i = (h_[bsl] @ Wh.T + b).astype(np.float32)      # [BC, D]
        baseT_i = np.ascontiguousarray(
            base_i.T.reshape(NKC, P, BC).transpose(1, 0, 2))
        # per-core: [NG, SG, NQ, 2, NEC, P] -> [NG, NQ, P, 2, NEC, SG]
        eoT_i = np.ascontiguousarray(eo_r[:, :, i].transpose(0, 2, 5, 3, 4, 1))
        in_maps.append(
            {"eoT": eoT_i, "weT": weT, "baseT": baseT_i, "vm": vm}
        )

    try:
        res = run_bass_kernel_spmd(nc, in_maps, list(range(NCORES)))
    except Exception:
        # transient NRT/device hiccups happen; one retry
        res = run_bass_kernel_spmd(nc, in_maps, list(range(NCORES)))
    global _last_results
    _last_results = res

    out = np.empty((B, S), dtype=np.float32)
    for i in range(NCORES):
        exps = np.asarray(res.results[i]["out"]).astype(np.float32)   # [128, S]
        sums = np.asarray(res.results[i]["esums"]).astype(np.float64)  # [128, NG]
        for bb in range(BC):
            row = 32 * (bb % 4) + bb // 4
            denom = np.float32(sums[row].sum())
            out[i * BC + bb] = exps[row] / denom
    return out


_last_results = None


if __name__ == "__main__":
    rng = np.random.default_rng(0)
    inputs = {
        "hidden": rng.standard_normal((1, B, D), dtype=np.float32),
        "encoder_outputs": rng.standard_normal((S, B, E2), dtype=np.float32),
        "W": (rng.standard_normal((D, E2 + D)) * 0.02).astype(np.float32),
        "b": (rng.standard_normal((D,)) * 0.02).astype(np.float32),
        "v": rng.random((D,), dtype=np.float32),
    }
    out = kernel(**inputs)
    print("out", out.shape, out.dtype, out.sum())
